# revision 6
# baseline (speedup 1.0000x reference)
"""Trainium2 Bass kernel for the sliding-window CNN problem.

Computes, for x[B=32, WORDS=512, E=256], W[1024, 1280], b[1024]:
    z[b,t,h] = sum_{w<5, e<256} x[b, t+w, e] * W[h, w*256+e]   (T = 508 windows)
    out[b,h] = relu(max_t z[b,t,h] + b[h])

Strategy: data-parallel over batch (4 batches per core, 8 cores).
Per core the window conv is 10 accumulating matmuls (5 window shifts x 2
feature chunks of 128) per [128h x 508t] PSUM tile; the window shift is a
free SBUF column offset on the moving operand.  fp16 operands (same PE
rate as bf16, ~8x better accuracy), fp32 PSUM accumulation.  Loop is
kc-outer over all 8 hidden chunks (8 PSUM banks in flight) so the PE's
weight-consumption rate stays below the DMA delivery rate and compute
overlaps the weight loads.  Max over time on DVE, bias+relu fused on
ScalarE, per-batch DMA out.

Startup: the framework preamble (all-engine barrier) ends at ~5.7-6.1us
per engine; HWDGE queues are per-engine, so the first input DMAs are
issued from the engines that exit the barrier earliest (vector/gpsimd/
scalar) in first-consumed order, and the PE warms the HAM clock gate
with a handful of junk matmuls only until the first operands land.
Tail: the last group's T columns are split in two chunks so the final
reduce covers half the columns, with relu(max(..)+bias) folded into one
tensor_scalar whose second operand is the first chunk's finished result.
"""

import numpy as np

import concourse.bacc as bacc
import concourse.mybir as mybir
import concourse.tile as tile
from concourse.bass_utils import run_bass_kernel_spmd

B, WORDS, E = 32, 512, 256
WIN = 5
HIDDEN = 1024
T = WORDS - WIN + 1          # 508 sliding windows
NCORES = 8
BPC = B // NCORES            # 4 batches per core
F = WIN * E                  # 1280 contraction features
KC = F // 128                # 10 contraction chunks
HC = HIDDEN // 128           # 8 hidden chunks
EC = E // 128                # 2 feature chunks per window position

FP16 = mybir.dt.float16
FP32 = mybir.dt.float32

_CACHE = {}


def _build():
    nc = bacc.Bacc(None, target_bir_lowering=False)
    # xT[p, b, ec, t] = x[b, t, ec*128+p]
    xT = nc.dram_tensor("xT", [128, BPC, EC, WORDS], FP16, kind="ExternalInput")
    # wT[p, kc, h] = W[h, kc*128+p]
    wT = nc.dram_tensor("wT", [128, KC, HIDDEN], FP16, kind="ExternalInput")
    bias = nc.dram_tensor("bias", [128, HC], FP32, kind="ExternalInput")
    # out[b, p, hc] = result for batch b, hidden unit hc*128+p
    out = nc.dram_tensor("out", [BPC, 128, HC], FP32, kind="ExternalOutput")

    with tile.TileContext(nc) as tc:
        with (
            tc.tile_pool(name="xin", bufs=1) as xpool,
            tc.tile_pool(name="wgt", bufs=1) as wpool,
            tc.tile_pool(name="ps", bufs=1, space="PSUM") as pspool,
            tc.tile_pool(name="post", bufs=2) as postpool,
            tc.tile_pool(name="cst", bufs=1) as cstpool,
        ):
            # Input DMAs, spread over the per-engine HWDGE queues in the
            # order the engines exit the framework preamble barrier
            # (vector ~5.65us, gpsimd ~5.70, scalar ~5.75, sync ~6.1) and
            # in consumption order: the first kc round needs wt[0] and
            # xt[0]'s first half; wt[kc] is consumed every ~1.7us after.
            xt = [
                xpool.tile([128, EC * WORDS], FP16, tag=f"x_{b}", name=f"x_{b}")
                for b in range(BPC)
            ]
            wt = [
                wpool.tile([128, HIDDEN], FP16, tag=f"w_{kc}", name=f"w_{kc}")
                for kc in range(KC)
            ]
            bias_sb = cstpool.tile([128, HC], FP32, tag="bias")

            junk = cstpool.tile([128, 128], FP16, tag="junk")
            nc.gpsimd.memset(junk[:], 0.0)
            nc.gpsimd.dma_start(xt[0][:, 0:WORDS], xT[:, 0, 0])
            nc.scalar.dma_start(wt[0][:, 0:512], wT[:, 0, 0:512])
            nc.sync.dma_start(wt[2][:], wT[:, 2])
            nc.gpsimd.dma_start(xt[0][:, WORDS:2 * WORDS], xT[:, 0, 1])
            nc.scalar.dma_start(wt[0][:, 512:HIDDEN], wT[:, 0, 512:HIDDEN])
            nc.sync.dma_start(wt[5][:], wT[:, 5])
            nc.gpsimd.dma_start(wt[3][:], wT[:, 3])
            nc.scalar.dma_start(wt[1][:], wT[:, 1])
            nc.sync.dma_start(wt[8][:], wT[:, 8])
            nc.gpsimd.dma_start(wt[6][:], wT[:, 6])
            nc.scalar.dma_start(wt[4][:], wT[:, 4])
            nc.sync.dma_start(wt[9][:], wT[:, 9])
            nc.gpsimd.dma_start(xt[1][:], xT[:, 1])
            nc.scalar.dma_start(wt[7][:], wT[:, 7])
            nc.sync.dma_start(xt[2][:], xT[:, 2])
            nc.gpsimd.dma_start(xt[3][:], xT[:, 3])
            nc.scalar.dma_start(bias_sb[:], bias[:])

            # PE pre-warm: the HAM clock gate holds the PE at 1.2 GHz until
            # it has seen ~3.4us of sustained activity.  The first operands
            # land ~2.2us after the tensor engine exits the preamble, so a
            # few junk matmuls (never-written SBUF, dead PSUM range) keep
            # the activity window busy until then; the first real matmuls
            # finish the warmup at half rate.
            ps_junk = pspool.tile([128, T], FP32, tag="ps7", name="ps_junk")
            for _ in range(20):
                nc.tensor.matmul(
                    ps_junk[:, 0:128], junk[:], junk[:],
                    start=True, stop=True,
                )

            def emit_group(b, hc, ps, c0, c1):
                """All KC accumulating matmuls for psum group (b, hc),
                moving-operand columns [c0:c1]."""
                for kc in range(KC):
                    w, ec = divmod(kc, EC)
                    base = ec * WORDS + w
                    nc.tensor.matmul(
                        ps[:, c0:c1],
                        wt[kc][:, hc * 128:(hc + 1) * 128],
                        xt[b][:, base + c0: base + c1],
                        start=(kc == 0),
                        stop=(kc == KC - 1),
                    )

            def emit_post(b, hc, ps, res):
                mx = postpool.tile([128, 1], FP32, tag=f"mx{hc}", name=f"mx_{b}_{hc}")
                nc.vector.reduce_max(mx[:], ps[:], axis=mybir.AxisListType.X)
                nc.scalar.activation(
                    res[:, hc:hc + 1], mx[:],
                    mybir.ActivationFunctionType.Relu,
                    bias=bias_sb[:, hc:hc + 1],
                )

            for b in range(BPC - 1):
                # kc-outer: all 8 banks accumulate in parallel; the PE's
                # weight consumption rate stays below DMA delivery, so
                # compute starts as soon as wt[0] lands.
                ps = [
                    pspool.tile([128, T], FP32, tag=f"ps{hc}", name=f"ps_{b}_{hc}")
                    for hc in range(HC)
                ]
                res = postpool.tile([128, HC], FP32, tag="res", name=f"res_{b}")
                for kc in range(KC):
                    w, ec = divmod(kc, EC)
                    base = ec * WORDS + w
                    rhs = xt[b][:, base: base + T]
                    for hc in range(HC):
                        nc.tensor.matmul(
                            ps[hc][:],
                            wt[kc][:, hc * 128:(hc + 1) * 128],
                            rhs,
                            start=(kc == 0),
                            stop=(kc == KC - 1),
                        )
                for hc in range(HC):
                    emit_post(b, hc, ps[hc], res)
                nc.sync.dma_start(out[b], res[:])

            # Last batch: hc-outer so groups finish staggered and the
            # reduce/act chain overlaps the remaining matmuls.  The final
            # group's T columns are split in two chunks, each fully
            # kc-accumulated, so only a half-width reduce plus one fused
            # tensor_scalar trails the last matmul; out ships in slices so
            # only 512B of DMA (issued from the vector engine itself)
            # trails the final op.
            b = BPC - 1
            res = postpool.tile([128, HC], FP32, tag="res", name="res_last")
            for hc in range(HC - 1):
                psl = pspool.tile([128, T], FP32, tag=f"ps{hc}", name=f"ps_l_{hc}")
                emit_group(b, hc, psl, 0, T)
                emit_post(b, hc, psl, res)
                if hc == 3:
                    nc.sync.dma_start(out[b, :, 0:4], res[:, 0:4])
            hc = HC - 1
            TA = 254                     # first chunk of the final group
            psl7 = pspool.tile([128, T], FP32, tag="ps7", name="ps_l_7")
            emit_group(b, hc, psl7, 0, TA)
            nc.sync.dma_start(out[b, :, 4:HC - 1], res[:, 4:HC - 1])
            mxa = postpool.tile([128, 1], FP32, tag="mxa", name="mxa_last")
            pre = postpool.tile([128, 1], FP32, tag="pre", name="pre_last")
            nc.vector.reduce_max(mxa[:], psl7[:, 0:TA], axis=mybir.AxisListType.X)
            nc.scalar.activation(
                pre[:], mxa[:],
                mybir.ActivationFunctionType.Relu,
                bias=bias_sb[:, hc:hc + 1],
            )
            emit_group(b, hc, psl7, TA, T)
            mxb = postpool.tile([128, 1], FP32, tag="mxb", name="mxb_last")
            nc.vector.reduce_max(mxb[:], psl7[:, TA:T], axis=mybir.AxisListType.X)
            # res = max(mxb + bias, relu(mxa + bias)) == relu(max_t z + bias)
            nc.vector.tensor_scalar(
                res[:, hc:hc + 1], mxb[:], bias_sb[:, hc:hc + 1], pre[:],
                mybir.AluOpType.add, mybir.AluOpType.max,
            )
            nc.scalar.dma_start(out[b, :, hc:hc + 1], res[:, hc:hc + 1])
    nc.finalize()
    return nc


def _prep(input, W, b):
    x = np.asarray(input, dtype=np.float32)
    # x[b, t, e] -> xT[p, b, ec, t] = x[b, t, ec*128+p]
    y = x.transpose(2, 0, 1).reshape(EC, 128, B, WORDS)      # [ec, p, b, t]
    xT = np.ascontiguousarray(y.transpose(1, 2, 0, 3)).astype(np.float16)  # [p,b,ec,t]
    # W[h, f] -> wT[p, kc, h] = W[h, kc*128+p]
    wt = np.asarray(W, dtype=np.float32).T.reshape(KC, 128, HIDDEN)  # [kc, p, h]
    wT = np.ascontiguousarray(wt.transpose(1, 0, 2)).astype(np.float16)  # [p, kc, h]
    # b[h] -> bias[p, hc] = b[hc*128+p]
    bias = np.ascontiguousarray(np.asarray(b, np.float32).reshape(HC, 128).T)
    return xT, wT, bias


def run(inputs, trace=False, **kwargs):
    if "nc" not in _CACHE:
        _CACHE["nc"] = _build()
    nc = _CACHE["nc"]
    xT, wT, bias = _prep(inputs["input"], inputs["W"], inputs["b"])
    in_maps = [
        {"xT": xT[:, c * BPC:(c + 1) * BPC], "wT": wT, "bias": bias}
        for c in range(NCORES)
    ]
    in_maps = [{k: np.ascontiguousarray(v) for k, v in m.items()} for m in in_maps]
    res = run_bass_kernel_spmd(nc, in_maps, list(range(NCORES)), trace=trace, **kwargs)
    # out[b, p, hc] -> full[core*BPC + b, hc*128 + p]
    parts = []
    for c in range(NCORES):
        o = res.results[c]["out"]              # [BPC, 128, HC]
        parts.append(o.transpose(0, 2, 1).reshape(BPC, HIDDEN))
    full = np.concatenate(parts, axis=0).astype(np.float32)
    return full, res


def kernel(**inputs):
    out, _ = run(inputs, trace=False)
    return out


# revision 9
# speedup vs baseline: 1.0810x; 1.0810x over previous
"""Trainium2 Bass kernel for the sliding-window CNN problem.

Computes, for x[B=32, WORDS=512, E=256], W[1024, 1280], b[1024]:
    z[b,t,h] = sum_{w<5, e<256} x[b, t+w, e] * W[h, w*256+e]   (T = 508 windows)
    out[b,h] = relu(max_t z[b,t,h] + b[h])

Strategy: data-parallel over batch (4 batches per core, 8 cores).
Per core the window conv is 10 accumulating matmuls (5 window shifts x 2
feature chunks of 128) per [128h x 508t] PSUM tile; the window shift is a
free SBUF column offset on the moving operand.  fp16 operands (same PE
rate as bf16, ~8x better accuracy), fp32 PSUM accumulation.  Loop is
kc-outer over all 8 hidden chunks (8 PSUM banks in flight) so the PE's
weight-consumption rate stays below the DMA delivery rate and compute
overlaps the weight loads.  Max over time on DVE, bias+relu fused on
ScalarE, per-batch DMA out.

Startup: the framework preamble (all-engine barrier) ends at ~5.7-6.1us
per engine; HWDGE queues are per-engine, so the first input DMAs are
issued from the engines that exit the barrier earliest (vector/gpsimd/
scalar) in first-consumed order, and the PE warms the HAM clock gate
with a handful of junk matmuls only until the first operands land.
Tail: the last group's T columns are split in two chunks so the final
reduce covers half the columns, with relu(max(..)+bias) folded into one
tensor_scalar whose second operand is the first chunk's finished result.
"""

import numpy as np

import concourse.bacc as bacc
import concourse.mybir as mybir
import concourse.tile as tile
from concourse.bass_utils import run_bass_kernel_spmd

B, WORDS, E = 32, 512, 256
WIN = 5
HIDDEN = 1024
T = WORDS - WIN + 1          # 508 sliding windows
NCORES = 8
BPC = B // NCORES            # 4 batches per core
F = WIN * E                  # 1280 contraction features
KC = F // 128                # 10 contraction chunks
HC = HIDDEN // 128           # 8 hidden chunks
EC = E // 128                # 2 feature chunks per window position

FP16 = mybir.dt.float16
FP32 = mybir.dt.float32

_CACHE = {}


def _build():
    nc = bacc.Bacc(None, target_bir_lowering=False)
    # xT[p, b, ec, t] = x[b, t, ec*128+p]
    xT = nc.dram_tensor("xT", [128, BPC, EC, WORDS], FP16, kind="ExternalInput")
    # wT[p, kc, h] = W[h, kc*128+p]
    wT = nc.dram_tensor("wT", [128, KC, HIDDEN], FP16, kind="ExternalInput")
    bias = nc.dram_tensor("bias", [128, HC], FP32, kind="ExternalInput")
    # out[b, p, hc] = result for batch b, hidden unit hc*128+p
    out = nc.dram_tensor("out", [BPC, 128, HC], FP32, kind="ExternalOutput")

    with tile.TileContext(nc) as tc:
        with (
            tc.tile_pool(name="xin", bufs=1) as xpool,
            tc.tile_pool(name="wgt", bufs=1) as wpool,
            tc.tile_pool(name="ps", bufs=1, space="PSUM") as pspool,
            tc.tile_pool(name="post", bufs=2) as postpool,
            tc.tile_pool(name="cst", bufs=1) as cstpool,
        ):
            # Input DMAs, spread over the per-engine HWDGE queues in the
            # order the engines exit the framework preamble barrier
            # (vector ~5.65us, gpsimd ~5.70, scalar ~5.75, sync ~6.1) and
            # in consumption order: the first kc round needs wt[0] and
            # xt[0]'s first half; wt[kc] is consumed every ~1.7us after.
            xt = [
                xpool.tile([128, EC * WORDS], FP16, tag=f"x_{b}", name=f"x_{b}")
                for b in range(BPC)
            ]
            wt = [
                wpool.tile([128, HIDDEN], FP16, tag=f"w_{kc}", name=f"w_{kc}")
                for kc in range(KC)
            ]
            bias_sb = cstpool.tile([128, HC], FP32, tag="bias")

            junk = cstpool.tile([128, 128], FP16, tag="junk")
            nc.vector.memset(junk[:], 0.0)
            nc.scalar.dma_start(wt[0][:, 0:512], wT[:, 0, 0:512])
            nc.sync.dma_start(xt[0][:, 0:WORDS], xT[:, 0, 0])
            nc.scalar.dma_start(wt[0][:, 512:HIDDEN], wT[:, 0, 512:HIDDEN])
            nc.sync.dma_start(xt[0][:, WORDS:2 * WORDS], xT[:, 0, 1])
            nc.scalar.dma_start(wt[1][:], wT[:, 1])
            nc.sync.dma_start(wt[2][:], wT[:, 2])
            nc.scalar.dma_start(wt[3][:], wT[:, 3])
            nc.sync.dma_start(wt[4][:], wT[:, 4])
            nc.scalar.dma_start(wt[5][:], wT[:, 5])
            nc.sync.dma_start(wt[6][:], wT[:, 6])
            nc.scalar.dma_start(wt[7][:], wT[:, 7])
            nc.sync.dma_start(wt[8][:], wT[:, 8])
            nc.scalar.dma_start(bias_sb[:], bias[:])
            nc.sync.dma_start(wt[9][:], wT[:, 9])
            nc.sync.dma_start(xt[1][:], xT[:, 1])
            nc.sync.dma_start(xt[2][:], xT[:, 2])
            nc.sync.dma_start(xt[3][:], xT[:, 3])

            # PE pre-warm: the HAM clock gate holds the PE at 1.2 GHz until
            # it has seen ~3.4us of sustained activity.  The first operands
            # land ~2.2us after the tensor engine exits the preamble, so a
            # few junk matmuls (never-written SBUF, dead PSUM range) keep
            # the activity window busy until then; the first real matmuls
            # finish the warmup at half rate.
            ps_junk = pspool.tile([128, T], FP32, tag="ps7", name="ps_junk")
            for _ in range(26):
                nc.tensor.matmul(
                    ps_junk[:, 0:128], junk[:], junk[:],
                    start=True, stop=True,
                )

            def emit_group(b, hc, ps, c0, c1):
                """All KC accumulating matmuls for psum group (b, hc),
                moving-operand columns [c0:c1]."""
                for kc in range(KC):
                    w, ec = divmod(kc, EC)
                    base = ec * WORDS + w
                    nc.tensor.matmul(
                        ps[:, c0:c1],
                        wt[kc][:, hc * 128:(hc + 1) * 128],
                        xt[b][:, base + c0: base + c1],
                        start=(kc == 0),
                        stop=(kc == KC - 1),
                    )

            def emit_post(b, hc, ps, res):
                mx = postpool.tile([128, 1], FP32, tag=f"mx{hc}", name=f"mx_{b}_{hc}")
                nc.vector.reduce_max(mx[:], ps[:], axis=mybir.AxisListType.X)
                nc.scalar.activation(
                    res[:, hc:hc + 1], mx[:],
                    mybir.ActivationFunctionType.Relu,
                    bias=bias_sb[:, hc:hc + 1],
                )

            for b in range(BPC - 1):
                # kc-outer: all 8 banks accumulate in parallel; the PE's
                # weight consumption rate stays below DMA delivery, so
                # compute starts as soon as wt[0] lands.
                ps = [
                    pspool.tile([128, T], FP32, tag=f"ps{hc}", name=f"ps_{b}_{hc}")
                    for hc in range(HC)
                ]
                res = postpool.tile([128, HC], FP32, tag="res", name=f"res_{b}")
                for kc in range(KC):
                    w, ec = divmod(kc, EC)
                    base = ec * WORDS + w
                    rhs = xt[b][:, base: base + T]
                    for hc in range(HC):
                        nc.tensor.matmul(
                            ps[hc][:],
                            wt[kc][:, hc * 128:(hc + 1) * 128],
                            rhs,
                            start=(kc == 0),
                            stop=(kc == KC - 1),
                        )
                for hc in range(HC):
                    emit_post(b, hc, ps[hc], res)
                nc.sync.dma_start(out[b], res[:])

            # Last batch: hc-outer so groups finish staggered and the
            # reduce/act chain overlaps the remaining matmuls.  The final
            # group's T columns are split in two chunks, each fully
            # kc-accumulated, so only a half-width reduce plus one fused
            # tensor_scalar trails the last matmul; out ships in slices so
            # only 512B of DMA (issued from the vector engine itself)
            # trails the final op.
            b = BPC - 1
            res = postpool.tile([128, HC], FP32, tag="res", name="res_last")
            for hc in range(HC - 1):
                psl = pspool.tile([128, T], FP32, tag=f"ps{hc}", name=f"ps_l_{hc}")
                emit_group(b, hc, psl, 0, T)
                emit_post(b, hc, psl, res)
                if hc == 3:
                    nc.sync.dma_start(out[b, :, 0:4], res[:, 0:4])
            hc = HC - 1
            TA = 254                     # first chunk of the final group
            psl7 = pspool.tile([128, TA], FP32, tag="ps7", name="ps_l_7a")
            emit_group(b, hc, psl7, 0, TA)
            nc.sync.dma_start(out[b, :, 4:HC - 1], res[:, 4:HC - 1])
            mxa = postpool.tile([128, 1], FP32, tag="mxa", name="mxa_last")
            pre = postpool.tile([128, 1], FP32, tag="pre", name="pre_last")
            nc.vector.reduce_max(mxa[:], psl7[:], axis=mybir.AxisListType.X)
            nc.scalar.activation(
                pre[:], mxa[:],
                mybir.ActivationFunctionType.Relu,
                bias=bias_sb[:, hc:hc + 1],
            )
            # chunk B goes in a different PSUM bank: a start=True matmul
            # clears the whole bank, so reusing chunk A's bank would
            # serialize chunk B behind chunk A's reduce.  Bank ps0 (last
            # used by this batch's hc=0 group) has long been reduced.
            pslb = pspool.tile([128, T - TA], FP32, tag="ps0", name="ps_l_7b")
            for kc in range(KC):
                w, ec = divmod(kc, EC)
                base = ec * WORDS + w
                nc.tensor.matmul(
                    pslb[:],
                    wt[kc][:, hc * 128:(hc + 1) * 128],
                    xt[b][:, base + TA: base + T],
                    start=(kc == 0),
                    stop=(kc == KC - 1),
                )
            mxb = postpool.tile([128, 1], FP32, tag="mxb", name="mxb_last")
            nc.vector.reduce_max(mxb[:], pslb[:], axis=mybir.AxisListType.X)
            # res = max(mxb + bias, relu(mxa + bias)) == relu(max_t z + bias)
            nc.vector.tensor_scalar(
                res[:, hc:hc + 1], mxb[:], bias_sb[:, hc:hc + 1], pre[:],
                mybir.AluOpType.add, mybir.AluOpType.max,
            )
            nc.scalar.dma_start(out[b, :, hc:hc + 1], res[:, hc:hc + 1])
    nc.finalize()
    return nc


def _prep(input, W, b):
    x = np.asarray(input, dtype=np.float32)
    # x[b, t, e] -> xT[p, b, ec, t] = x[b, t, ec*128+p]
    y = x.transpose(2, 0, 1).reshape(EC, 128, B, WORDS)      # [ec, p, b, t]
    xT = np.ascontiguousarray(y.transpose(1, 2, 0, 3)).astype(np.float16)  # [p,b,ec,t]
    # W[h, f] -> wT[p, kc, h] = W[h, kc*128+p]
    wt = np.asarray(W, dtype=np.float32).T.reshape(KC, 128, HIDDEN)  # [kc, p, h]
    wT = np.ascontiguousarray(wt.transpose(1, 0, 2)).astype(np.float16)  # [p, kc, h]
    # b[h] -> bias[p, hc] = b[hc*128+p]
    bias = np.ascontiguousarray(np.asarray(b, np.float32).reshape(HC, 128).T)
    return xT, wT, bias


def run(inputs, trace=False, **kwargs):
    if "nc" not in _CACHE:
        _CACHE["nc"] = _build()
    nc = _CACHE["nc"]
    xT, wT, bias = _prep(inputs["input"], inputs["W"], inputs["b"])
    in_maps = [
        {"xT": xT[:, c * BPC:(c + 1) * BPC], "wT": wT, "bias": bias}
        for c in range(NCORES)
    ]
    in_maps = [{k: np.ascontiguousarray(v) for k, v in m.items()} for m in in_maps]
    res = run_bass_kernel_spmd(nc, in_maps, list(range(NCORES)), trace=trace, **kwargs)
    # out[b, p, hc] -> full[core*BPC + b, hc*128 + p]
    parts = []
    for c in range(NCORES):
        o = res.results[c]["out"]              # [BPC, 128, HC]
        parts.append(o.transpose(0, 2, 1).reshape(BPC, HIDDEN))
    full = np.concatenate(parts, axis=0).astype(np.float32)
    return full, res


def kernel(**inputs):
    out, _ = run(inputs, trace=False)
    return out
